# revision 17
# baseline (speedup 1.0000x reference)
"""Trainium2 Bass kernel for CoordLSVotingWeighted (segment_reduce).

Strategy: data-parallel over batch B=8 across 8 NeuronCores (1 image/core).
Host repacks inputs (pure layout, no math): w full, [dy|dx] per half, seg
full, [cw|ch] constants. 5 large DMAs keep the stream bandwidth-bound.
Per image, on device:
  scalar : ew=Exp(w); sqx=Square(dx); sp=Ln(ew+1) (one table switch);
           hotch = hot*ch (Copy with per-partition scale)
  gpsimd : sqy=dy*dy, pp=dx*dy per quarter; R pad memset
  vector : mx=max_c(seg), hot=(seg==mx), hotcw=hot*cw per half; per quarter:
           s=sqx+sqy, rinv~=1/s (custom approx), k=sp*rinv, R00=k*sqy,
           R11=k*sqx, m=k*pp
  tensor : 32 block-diagonal matmuls psum[96,120] += L[:,g]^T @ R[:,g]
           L packed per w: [hot(8)|hot*ch(8)|hot*cw(8)]
           R packed per w: [R00(9) pad|R11(9) pad|m(9) pad]
Host: contract the w-diagonal, assemble 2x2 systems in float64, pinv-solve,
scale by HEIGHT.

Self-contained: only needs numpy / ml_dtypes / concourse (installed env).
"""

import os

import numpy as np

B = 8
H = 128
W = 128
NCLS = 9  # seg channels, class 0 = background
NPTS = 9
OC = 8
HEIGHT = 128.0
N_CORES = 8
NF = W * NPTS  # 1152
KC = 4  # w-columns per matmul group
NG = W // KC  # 32 matmul groups
HW_ = W // 2  # half width
QW = W // 4  # quarter width
HF = HW_ * NPTS  # 576

_cache: dict = {}


def _build_nc():
    import concourse.bacc as bacc
    import concourse.tile as tile
    import concourse.mybir as mybir
    from concourse.alu_op_type import AluOpType as Alu

    Act = mybir.ActivationFunctionType
    Axis = mybir.AxisListType
    f32 = mybir.dt.float32
    b16 = mybir.dt.bfloat16

    nc = bacc.Bacc(
        "TRN2", target_bir_lowering=False, debug=False, num_devices=N_CORES
    )
    w_d = nc.dram_tensor("w", [H, NF], f32, kind="ExternalInput")
    dd_d = [
        nc.dram_tensor(f"dd{h}", [H, 2 * HF], f32, kind="ExternalInput")
        for h in range(2)
    ]  # [dy_half | dx_half]
    seg_d = nc.dram_tensor("seg", [H, W * NCLS], f32, kind="ExternalInput")
    cw_d = nc.dram_tensor("cwch", [H, OC * W + 1], b16, kind="ExternalInput")
    out_d = nc.dram_tensor("acc", [24 * KC, 2 * 30 * KC], f32, kind="ExternalOutput")

    s8 = [(0, HW_ * OC), (HW_ * OC, OC * W)]  # (w c) half slices
    s9 = [(0, HF), (HF, NF)]  # (w p) half slices
    s1 = [(0, HW_), (HW_, W)]  # per-w half slices
    q9 = [(j * QW * NPTS, (j + 1) * QW * NPTS) for j in range(4)]
    q1 = [(j * QW, (j + 1) * QW) for j in range(4)]

    with tile.TileContext(nc) as tc:
        with (
            tc.tile_pool(name="main", bufs=1) as pool,
            tc.tile_pool(name="ps", bufs=1, space="PSUM") as psp,
        ):
            # ---- input tiles
            wdt = pool.tile([H, NF], f32, tag="wdt")
            ddt0 = pool.tile([H, 2 * HF], f32, tag="ddt0")
            ddt1 = pool.tile([H, 2 * HF], f32, tag="ddt1")
            ddt = [ddt0, ddt1]
            sgt = pool.tile([H, W * NCLS], f32, tag="sgt")
            cwt = pool.tile([H, OC * W + 1], b16, tag="cwt")

            nc.sync.dma_start(out=wdt[:, :], in_=w_d[:, :])
            nc.sync.dma_start(out=sgt[:, :], in_=seg_d[:, :])
            nc.sync.dma_start(out=ddt[0][:, :], in_=dd_d[0][:, :])
            nc.sync.dma_start(out=ddt[1][:, :], in_=dd_d[1][:, :])
            nc.sync.dma_start(out=cwt[:, :], in_=cw_d[:, :])

            def dyv(h, a, b):  # dy slice within half tile (local offsets)
                return ddt[h][:, a:b]

            def dxv(h, a, b):
                return ddt[h][:, HF + a : HF + b]

            # ---- work tiles
            ew16 = pool.tile([H, NF], b16, tag="ew16")
            sp16 = pool.tile([H, NF], b16, tag="sp16")
            sqx = pool.tile([H, NF], b16, tag="sqx")
            sqy = pool.tile([H, NF], b16, tag="sqy")
            pp = pool.tile([H, NF], b16, tag="pp")
            s32 = pool.tile([H, NF], f32, tag="s32")
            rinv = pool.tile([H, NF], f32, tag="rinv")
            k16 = pool.tile([H, NF], b16, tag="k16")
            mx = pool.tile([H, W], f32, tag="mx")
            # L packed per w: [hot(8) | hot*ch(8) | hot*cw(8)]
            L = pool.tile([H, W * 24], b16, tag="L")
            # R packed per w: [R00(9) pad | R11(9) pad | m(9) pad]
            R = pool.tile([H, W * 30], b16, tag="R")

            cht32 = pool.tile([H, 1], f32, tag="cht32")
            nc.vector.tensor_copy(
                out=cht32[:, :], in_=cwt[:, OC * W : OC * W + 1]
            )

            sgt_wc = sgt[:, :].rearrange("q (w c) -> q w c", c=NCLS)
            L_w = L[:, :].rearrange("q (w x) -> q w x", x=24)
            R_w = R[:, :].rearrange("q (w f) -> q w f", f=30)
            nc.gpsimd.memset(R_w[:, :, 9:30:10], 0.0)

            # ---- scalar: Exp, Ln full width; one Square per dd tensor
            nc.scalar.activation(out=ew16[:, :], in_=wdt[:, :], func=Act.Exp)
            nc.scalar.activation(
                out=sp16[:, :], in_=ew16[:, :], func=Act.Ln, bias=1.0
            )
            for h in range(2):
                a9, b9_ = s9[h]
                nc.scalar.activation(
                    out=sqy[:, a9:b9_], in_=dyv(h, 0, HF), func=Act.Square
                )
                nc.scalar.activation(
                    out=sqx[:, a9:b9_], in_=dxv(h, 0, HF), func=Act.Square
                )

            # ---- vector: dx*dy per quarter (pp feeds the m feature)
            for j in range(4):
                h = j // 2
                la, lb = (0, HF // 2) if j % 2 == 0 else (HF // 2, HF)
                a9, b9_ = q9[j]
                nc.vector.tensor_tensor(
                    out=pp[:, a9:b9_], in0=dxv(h, la, lb), in1=dyv(h, la, lb),
                    op=Alu.mult,
                )

            # ---- vector: full-width hot path + feature chain per quarter
            nc.vector.tensor_reduce(
                out=mx[:, :], in_=sgt_wc, axis=Axis.X, op=Alu.max
            )
            mx_b = mx[:, :].unsqueeze(2).broadcast_to((H, W, OC))
            hot_f = L_w[:, :, 0:8]
            nc.vector.tensor_tensor(
                out=hot_f, in0=sgt_wc[:, :, 1:NCLS], in1=mx_b,
                op=Alu.is_equal,
            )
            nc.scalar.mul(L_w[:, :, 8:16], hot_f, cht32[:, :])
            cwt_wc = cwt[:, 0 : OC * W].rearrange("q (w c) -> q w c", c=OC)
            nc.vector.tensor_tensor(
                out=L_w[:, :, 16:24], in0=hot_f, in1=cwt_wc, op=Alu.mult
            )

            for h in range(2):
                for jj in range(2):
                    j = 2 * h + jj
                    a9, b9_ = q9[j]
                    a1q, b1q = q1[j]
                    nc.vector.tensor_tensor(
                        out=s32[:, a9:b9_], in0=sqx[:, a9:b9_],
                        in1=sqy[:, a9:b9_], op=Alu.add,
                    )
                    nc.vector.reciprocal_approx_fast(
                        out=rinv[:, a9:b9_], in_=s32[:, a9:b9_]
                    )
                    nc.vector.tensor_tensor(
                        out=k16[:, a9:b9_], in0=sp16[:, a9:b9_],
                        in1=rinv[:, a9:b9_], op=Alu.mult,
                    )
                    k_r = k16[:, a9:b9_].rearrange("q (w p) -> q w p", p=NPTS)
                    sqy_r = sqy[:, a9:b9_].rearrange(
                        "q (w p) -> q w p", p=NPTS
                    )
                    sqx_r = sqx[:, a9:b9_].rearrange(
                        "q (w p) -> q w p", p=NPTS
                    )
                    pp_r = pp[:, a9:b9_].rearrange("q (w p) -> q w p", p=NPTS)
                    nc.vector.tensor_tensor(
                        out=R_w[:, a1q:b1q, 0:9], in0=k_r, in1=sqy_r,
                        op=Alu.mult,
                    )
                    nc.vector.tensor_tensor(
                        out=R_w[:, a1q:b1q, 10:19], in0=k_r, in1=sqx_r,
                        op=Alu.mult,
                    )
                    nc.vector.tensor_tensor(
                        out=R_w[:, a1q:b1q, 20:29], in0=k_r, in1=pp_r,
                        op=Alu.mult,
                    )

            # ---- segment reduce: 32 block-diagonal accumulating matmuls,
            # one psum accumulator per half so evacuation overlaps compute
            acc0 = psp.tile([24 * KC, 30 * KC], f32, tag="acc0")
            acc1 = psp.tile([24 * KC, 30 * KC], f32, tag="acc1")
            accs = [acc0, acc1]
            outs = pool.tile([24 * KC, 2 * 30 * KC], f32, tag="outs")
            for hh in range(2):
                acc = accs[hh]
                for gg in range(NG // 2):
                    g = hh * (NG // 2) + gg
                    nc.tensor.matmul(
                        acc[:, :],
                        L[:, g * 24 * KC : (g + 1) * 24 * KC],
                        R[:, g * 30 * KC : (g + 1) * 30 * KC],
                        start=(gg == 0),
                        stop=(gg == NG // 2 - 1),
                    )
                nc.scalar.copy(
                    out=outs[:, hh * 30 * KC : (hh + 1) * 30 * KC],
                    in_=acc[:, :],
                )
                nc.sync.dma_start(
                    out=out_d[:, hh * 30 * KC : (hh + 1) * 30 * KC],
                    in_=outs[:, hh * 30 * KC : (hh + 1) * 30 * KC],
                )

    nc.compile()
    return nc


def _host_constants():
    import ml_dtypes

    bf16 = ml_dtypes.bfloat16
    coord = ((np.arange(W, dtype=np.float32) + 0.5) / HEIGHT).astype(bf16)
    cwc8 = np.broadcast_to(coord[:, None], (W, OC)).reshape(W * OC)
    chv = ((np.arange(H, dtype=np.float32) + 0.5) / HEIGHT).astype(bf16)
    cwch = np.empty((H, W * OC + 1), dtype=bf16)
    cwch[:, : W * OC] = cwc8[None, :]
    cwch[:, W * OC] = chv
    return cwch


def _solve_host(acc_f32: np.ndarray) -> np.ndarray:
    """acc [96,240] fp32 -> p [OC, NPTS, 2] fp32 (float64 pinv like ref)."""
    a6 = acc_f32.astype(np.float64).reshape(KC, 3, OC, 2, KC, 30)
    # contract the w-diagonal within each matmul group; cols per w:
    # [R00(9) pad | R11(9) pad | m(9) pad]
    tt = np.einsum("wtcawf->tcf", a6)  # [3, 8, 30]
    A = tt[0, :, 0:9]
    D = tt[0, :, 10:19]
    Bm = tt[0, :, 20:29]
    S1 = tt[1, :, 0:9]
    S3 = tt[1, :, 20:29]
    S2 = tt[2, :, 20:29]
    S4 = tt[2, :, 10:19]
    Rm = np.empty((OC, NPTS, 2, 2), dtype=np.float64)
    Rm[..., 0, 0] = A
    Rm[..., 0, 1] = -Bm
    Rm[..., 1, 0] = -Bm
    Rm[..., 1, 1] = D
    q = np.stack([S1 - S2, S4 - S3], axis=-1)
    Rp = np.linalg.pinv(Rm.reshape(-1, 2, 2)).reshape(Rm.shape)
    p = np.einsum("cpij,cpj->cpi", Rp, q) * HEIGHT
    return p.astype(np.float32)


def kernel(seg, direct, w):
    if "nc" not in _cache:
        _cache["nc"] = _build_nc()
    nc = _cache["nc"]

    seg = np.ascontiguousarray(np.asarray(seg, dtype=np.float32))
    direct = np.asarray(direct, dtype=np.float32)
    w = np.ascontiguousarray(np.asarray(w, dtype=np.float32))
    cwch = _host_constants()

    d4 = direct.reshape(B, H, W, NPTS, 2)
    dd = []
    for h in range(2):
        sl = slice(0, HW_) if h == 0 else slice(HW_, W)
        buf = np.empty((B, H, 2 * HF), dtype=np.float32)
        buf[:, :, :HF] = d4[:, :, sl, :, 1].reshape(B, H, HF)  # dy
        buf[:, :, HF:] = d4[:, :, sl, :, 0].reshape(B, H, HF)  # dx
        dd.append(buf)

    in_maps = []
    for i in range(B):
        in_maps.append(
            {
                "w": w[i].reshape(H, NF),
                "dd0": dd[0][i],
                "dd1": dd[1][i],
                "seg": seg[i].reshape(H, W * NCLS),
                "cwch": cwch,
            }
        )

    from concourse.bass_utils import run_bass_kernel_spmd

    trace = bool(int(os.environ.get("KERNEL_TRACE", "0")))
    res = run_bass_kernel_spmd(
        nc, in_maps, core_ids=list(range(N_CORES)), trace=trace
    )
    kernel._last_exec_ns = res.exec_time_ns
    kernel._last_results = res

    out = np.stack(
        [_solve_host(np.asarray(res.results[i]["acc"])) for i in range(B)],
        axis=0,
    )
    return out
